# revision 19
# baseline (speedup 1.0000x reference)
"""Trainium2 Bass kernel for nn_Attention_2851858284976.

Dense transformer attention block, b=8 n=1024 dim=1024 heads=16.
Sharding: pure data parallel — one batch element per NeuronCore (8 cores).

Per-core math (batch element x of shape (n, dim)):
  Y = x @ w_qkv^T                              (n, 3*dim)
  Z = Y.reshape(49152, 64)   # raw reshape: rows are (token, col-block) pairs
  Q = Z[0:16384], K = Z[16384:32768], V = Z[32768:49152], each (16, 1024, 64)
  per head: P^T = exp(scale * K_h @ Q_h^T)     (softmax along the partition axis)
            [O^T; Zs*64] = [V_h | 1*64]^T @ P^T  (ones cols replicate the denom)
            oT_h = O^T * (1/Zs)
  out = (oT stacked).T @ w_out^T + b_out

Q/K SBUF layout: Z row r = 48*i + c0 decomposes as c0 = 16*a + b (a<3,
b<16), r = 16*T + b with T = 3*i + a.  Heads are 1024 = 64*16 tokens, so
head h's tokens t = 16*u + b map to a contiguous T-run [64h, 64h+64) x all
b.  QKb[d, T, b] (b innermost) therefore gives:
  - contiguous head windows with IDENTITY token order (col = 16*dT + b);
  - phase-1A scatters that write [i, b-run] blocks (4 bf16 at 4B step)
    instead of 96B-strided single elements.

Schedule: the exp stream on ACT (128 x ~1.15us) is the second-largest
engine load after the PE, so phase 1 is split to start it early:
  A-sweep tb0 (i<384; all Q + K of heads 0-1) -> B-it5 (V heads 0-3) ->
  consume loop from head 0 with the REST of phase 1 (A tb1/tb2 2-block
  groups, B it6/it7) spliced between steps.  PSUM: sps 2x2 banks, ops 2
  banks (tags reused each head; [O;Z] spilled to SBUF f32 in one copy to
  free them), leaving 2 banks for the spliced phase-1 psums.
"""
import numpy as np
import ml_dtypes

import concourse.bass as bass
import concourse.mybir as mybir
from concourse import bacc
from concourse.tile import TileContext
from concourse.bass_utils import run_bass_kernel_spmd

N_CORES = 8
N = 1024          # tokens
DIM = 1024
E3 = 3 * DIM      # qkv projection width
H = 16            # heads
HD = 64           # head dim
SCALE = HD ** -0.5
TSPAN = 2049      # T = 3*i + a, i < 683 -> T in [0, 2049)
VROWS = 48 * 342  # 16416: V Z-rows (16384) plus 32 slack
AHEAD = 6         # produce-ahead depth (pt queue)

F32 = mybir.dt.float32
BF = mybir.dt.bfloat16
FT = mybir.ActivationFunctionType


def build():
    nc = bacc.Bacc("TRN2", target_bir_lowering=False, num_devices=N_CORES)
    xt = nc.declare_dram_parameter("xt", [DIM, N], BF, isOutput=False)
    wqkvt = nc.declare_dram_parameter("wqkvt", [DIM, E3], BF, isOutput=False)
    woutt = nc.declare_dram_parameter("woutt", [DIM, DIM], BF, isOutput=False)
    bias = nc.declare_dram_parameter("bias", [1, DIM], F32, isOutput=False)
    outp = nc.declare_dram_parameter("out", [N, DIM], F32, isOutput=True)

    with TileContext(nc) as tc:
        with tc.tile_pool(name="dram", bufs=1, space="DRAM") as dpool, \
             tc.tile_pool(name="singles", bufs=1) as singles:
            # packed V buffer: flat row q = Z row 32768+q = v of head q//1024
            vbuf = dpool.tile([VROWS, HD], BF)
            vb3 = vbuf.rearrange("(a c) d -> a c d", c=48)   # (342, 48, 64)

            oT = singles.tile([128, 8, N], BF)    # [64*(h%2)+dd, h//2, i]
            biasrep = singles.tile([128, DIM], F32)
            vh0 = singles.tile([128, 8, 2 * HD], BF)
            vh1 = singles.tile([128, 8, 2 * HD], BF)
            vh2 = singles.tile([128, 8, 2 * HD], BF)
            vh3 = singles.tile([128, 8, 2 * HD], BF)
            vhs = [vh0, vh1, vh2, vh3]
            WOT = singles.tile([128, 8, DIM], BF)

            with tc.tile_pool(name="qk", bufs=1) as qkpool:
                # QKb[d, T, b] = Z[16*T + b, d]  (Q: T<1024, K: 1024<=T<2048)
                QKb = qkpool.tile([64, TSPAN, 16], BF)
                # scatter view: T = 3*i + a, b = 2*b2 + dl
                QKs = QKb.rearrange("p (i a) (b2 dl) -> p i a b2 dl",
                                    a=3, dl=2)

                def qt_sl(h, ic):
                    w = QKb[0:64, 64 * h + 32 * ic:64 * h + 32 * ic + 32, :]
                    return w.rearrange("p T b -> p (T b)")

                def kt_sl(h, jt):
                    w = QKb[0:64, 1024 + 64 * h + 8 * jt:
                            1024 + 64 * h + 8 * jt + 8, :]
                    return w.rearrange("p T b -> p (T b)")

                with tc.tile_pool(name="p1", bufs=1) as p1:
                    XT = p1.tile([128, 8, N], BF)
                    # WT[:, kt, a, e'] = wqkvt rows kt*128.., col a*1024+e'
                    WT = p1.tile([128, 8, 3, N], BF)
                    for kt in range(8):
                        nc.sync.dma_start(
                            out=XT[:, kt, :], in_=xt[kt * 128:(kt + 1) * 128, :])
                    for a in range(3):
                        for kh in range(2):
                            nc.sync.dma_start(
                                out=WT[:, 4 * kh:4 * kh + 4, a, :],
                                in_=wqkvt[kh * 512:(kh + 1) * 512,
                                          a * N:(a + 1) * N].rearrange(
                                              "(kt p) e -> p kt e", p=128))
                    nc.sync.dma_start(out=biasrep,
                                      in_=bias[:].to_broadcast((128, DIM)))
                    nc.sync.dma_start(
                        out=WOT, in_=woutt[:].rearrange("(a p) e -> p a e", p=128))
                    # [V | ones*64] stationaries for the PV matmul; ones half
                    # replicates the softmax denominator on out rows 64-127.
                    for v in vhs:
                        nc.vector.memset(v, 1.0)

                    # sweep boundaries: tb0 covers all Q (i<=341) + K heads
                    # 0-1 (i<=383); tb1 unlocks K heads 2-7 (i<=511); tb2
                    # the rest.
                    SWEEPS = [(0, 384), (384, 534), (534, 683)]

                    def a_group_mms(ps, nblk, a, m0, i_lo, i_hi):
                        cnt = i_hi - i_lo
                        for mm in range(nblk):
                            m = m0 + mm
                            for kt in range(8):
                                nc.tensor.matmul(
                                    ps[:, mm, 0:cnt],
                                    lhsT=WT[:, kt, a, m * 128:(m + 1) * 128],
                                    rhs=XT[:, kt, i_lo:i_hi],
                                    start=(kt == 0), stop=(kt == 7))

                    def a_group_scatter(ps, nblk, a, m0, i_lo, i_hi):
                        cnt = i_hi - i_lo
                        for dl in range(2):  # c0 parity
                            src = ps[64 * dl:64 * dl + 64, :, 0:cnt]
                            src = src.rearrange("p m i -> p i m")
                            dst = QKs[0:64, i_lo:i_hi, a, m0:m0 + nblk, dl]
                            if dl == 0:
                                nc.scalar.copy(dst, src)
                            else:
                                nc.vector.tensor_copy(dst, src)

                    # ---------- phase 1A sweep tb0 (i < 384) --------------
                    with tc.tile_pool(name="psA0", bufs=2, space="PSUM") as psA0:
                        i_lo, i_hi = SWEEPS[0]
                        for a in range(3):
                            for mh in range(2):
                                ps4 = psA0.tile([128, 4, 512], F32)
                                a_group_mms(ps4, 4, a, 4 * mh, i_lo, i_hi)
                                a_group_scatter(ps4, 4, a, 4 * mh, i_lo, i_hi)

                    # ---------- phase 1B tiles ----------------------------
                    def emit_b_tile(bit, ec, pBpool, psBpool):
                        ps = psBpool.tile([128, 512], F32)
                        for kt in range(8):
                            nc.tensor.matmul(
                                ps,
                                lhsT=XT[:, kt, bit * 128:(bit + 1) * 128],
                                rhs=WT[:, kt, (ec * 512) // N,
                                       (ec * 512) % N:(ec * 512) % N + 512],
                                start=(kt == 0), stop=(kt == 7))
                        st = pBpool.tile([128, 8, HD], BF)
                        nc.vector.tensor_copy(
                            st, ps.rearrange("p (b d) -> p b d", d=HD))
                        # V rows: q = 48*T + c0 - 32768
                        if ec <= 3:
                            plo = 43 if bit == 5 else 0
                            nc.sync.dma_start(
                                out=vb3[bit * 128 + plo - 683:
                                        (bit + 1) * 128 - 683,
                                        ec * 8 + 16: ec * 8 + 24, :],
                                in_=st[plo:128, :, :])
                        else:
                            plo = 42 if bit == 5 else 0
                            nc.sync.dma_start(
                                out=vb3[bit * 128 + plo - 682:
                                        (bit + 1) * 128 - 682,
                                        (ec - 4) * 8: (ec - 4) * 8 + 8, :],
                                in_=st[plo:128, :, :])

                    def load_v(h):
                        nc.sync.dma_start(
                            out=vhs[h % 4][:, :, 0:HD],
                            in_=vbuf[h * N:(h + 1) * N, :].rearrange(
                                "(t p) d -> p t d", p=128))

                    with tc.tile_pool(name="pB", bufs=2) as pB:
                        # B-it5 first: V heads 0-3 live in its rows.
                        with tc.tile_pool(name="psB5", bufs=4,
                                          space="PSUM") as psB5:
                            for ec in range(6):
                                emit_b_tile(5, ec, pB, psB5)
                        for hv in range(4):
                            load_v(hv)

                        # ------ phase 2 + spliced phase-1 leftovers -------
                        with tc.tile_pool(name="pt", bufs=AHEAD) as ptpool, \
                             tc.tile_pool(name="sp", bufs=1) as sppool, \
                             tc.tile_pool(name="rz", bufs=1) as rzpool, \
                             tc.tile_pool(name="sps", bufs=2,
                                          space="PSUM") as spsum, \
                             tc.tile_pool(name="ops", bufs=1,
                                          space="PSUM") as opsum:
                            steps = [(h, jt) for h in range(H) for jt in range(8)]

                            def produce(h, jt):
                                sps = spsum.tile([128, 2, 512], F32, tag="sps")
                                warm = vhs[(h + 2) % 4]
                                # HAM warm-keeper; overwritten by the real
                                # scores matmul (start=True).
                                nc.tensor.matmul(
                                    sps[0:128, 0, 0:128],
                                    lhsT=warm[:, 0, :], rhs=warm[:, 0, :],
                                    start=True, stop=True)
                                for ic in range(2):
                                    nc.tensor.matmul(
                                        sps[:, ic, :],
                                        lhsT=kt_sl(h, jt),
                                        rhs=qt_sl(h, ic),
                                        start=True, stop=True)
                                pt = ptpool.tile([128, 2, 512], BF, tag="pt")
                                nc.scalar.activation(pt, sps, FT.Exp, scale=SCALE)
                                return pt

                            # emission-order gates (the tile framework can
                            # only order reads after ALREADY-EMITTED writes):
                            # produce(h) needs its K sweep scattered; V loads
                            # for h>=4 need the B it6/it7 stores emitted.
                            state = {"next_p": 0, "limit": 16, "ops": None,
                                     "pend_v": []}
                            pts = []

                            def fill_pts(s):
                                while (state["next_p"] < min(s + 1 + AHEAD,
                                                             state["limit"],
                                                             len(steps))):
                                    pts.append(produce(*steps[state["next_p"]]))
                                    state["next_p"] += 1

                            def flush_v(s):
                                keep = []
                                for thr, hv in state["pend_v"]:
                                    if s >= thr:
                                        load_v(hv)
                                    else:
                                        keep.append((thr, hv))
                                state["pend_v"] = keep

                            fill_pts(-1)

                            def consume_step(s):
                                h, jt = steps[s]
                                po, hf = 64 * (h % 2), h // 2
                                flush_v(s)
                                if jt == 0:
                                    # 2 tags reused by every head: frees 2
                                    # PSUM banks for the spliced phase-1 work
                                    ops0 = opsum.tile([128, 512], F32,
                                                      tag="ops0")
                                    ops1 = opsum.tile([128, 512], F32,
                                                      tag="ops1")
                                    state["ops"] = (ops0, ops1)
                                ops = state["ops"]
                                pt_cur = pts.pop(0)
                                fill_pts(s)
                                for ic in range(2):
                                    nc.tensor.matmul(
                                        ops[ic],
                                        lhsT=vhs[h % 4][:, jt, :],
                                        rhs=pt_cur[:, ic, :],
                                        start=(jt == 0), stop=(jt == 7),
                                        skip_group_check=True)
                                if jt == 7 and h + 4 < H:
                                    hv = h + 4
                                    thr = 0 if hv < 4 else (30 if hv < 10 else 36)
                                    if s >= thr:
                                        load_v(hv)
                                    else:
                                        state["pend_v"].append((thr, hv))
                                if jt == 7:
                                    # spill [O; Z] to SBUF to free the ops
                                    # tags fast; recip+mul run off the
                                    # critical path.  The custom-DVE recip
                                    # needs UNSHIFTED partitions (regular
                                    # copies handle the 64->0 shift).
                                    spz = sppool.tile([64, 2, 512], F32,
                                                      tag=f"spz{h % 2}")
                                    spo = sppool.tile([64, 2, 512], F32,
                                                      tag=f"spo{h % 2}")
                                    for ic in range(2):
                                        nc.vector.tensor_copy(
                                            spz[:, ic, :], ops[ic][64:128, :])
                                        nc.vector.tensor_copy(
                                            spo[:, ic, :], ops[ic][0:64, :])
                                    rzs = rzpool.tile([64, 2, 512], F32,
                                                      tag="rzs")
                                    nc.vector.reciprocal_approx_fast(rzs, spz)
                                    nc.vector.tensor_mul(
                                        oT[po:po + 64, hf, :].rearrange(
                                            "p (i c) -> p i c", c=512),
                                        spo, rzs)

                            s = 0
                            # A sweeps tb1/tb2 as 2-block groups (2 psum
                            # banks, bufs=1) spliced one per consume step.
                            # tb1 complete -> K heads 2-8 valid; tb2 -> all.
                            with tc.tile_pool(name="psA1", bufs=1,
                                              space="PSUM") as psA1:
                                for tb in (1, 2):
                                    i_lo, i_hi = SWEEPS[tb]
                                    for a in range(3):
                                        for mp in range(4):
                                            consume_step(s); s += 1
                                            ps2 = psA1.tile([128, 2, 512], F32)
                                            a_group_mms(ps2, 2, a, 2 * mp,
                                                        i_lo, i_hi)
                                            a_group_scatter(ps2, 2, a, 2 * mp,
                                                            i_lo, i_hi)
                                    state["limit"] = 72 if tb == 1 else 128
                                    fill_pts(s - 1)
                            with tc.tile_pool(name="psB2", bufs=2,
                                              space="PSUM") as psB2:
                                for bit in (6, 7):
                                    for ec in range(6):
                                        consume_step(s); s += 1
                                        emit_b_tile(bit, ec, pB, psB2)
                            while s < len(steps):
                                consume_step(s); s += 1

                # ---------- phase 3: out = oT.T @ w_out^T + b ----------
                with tc.tile_pool(name="p3st", bufs=4) as p3st, \
                     tc.tile_pool(name="ps3", bufs=4, space="PSUM") as ps3p:
                    for it in range(8):
                        for ec in range(2):
                            rps = ps3p.tile([128, 512], F32)
                            for ct in range(8):
                                nc.tensor.matmul(
                                    rps,
                                    lhsT=oT[:, ct, it * 128:(it + 1) * 128],
                                    rhs=WOT[:, ct, ec * 512:(ec + 1) * 512],
                                    start=(ct == 0), stop=(ct == 7))
                            ost = p3st.tile([128, 512], F32)
                            nc.vector.tensor_add(
                                ost, rps, biasrep[:, ec * 512:(ec + 1) * 512])
                            nc.sync.dma_start(
                                out=outp[it * 128:(it + 1) * 128,
                                         ec * 512:(ec + 1) * 512],
                                in_=ost)

    nc.finalize()
    return nc


_CACHE = {}


def _get_nc():
    if "nc" not in _CACHE:
        _CACHE["nc"] = build()
    return _CACHE["nc"]


def make_in_maps(x, w_qkv, w_out, b_out):
    bf = ml_dtypes.bfloat16
    wqkvt = np.ascontiguousarray(np.asarray(w_qkv, dtype=np.float32).T).astype(bf)
    woutt = np.ascontiguousarray(np.asarray(w_out, dtype=np.float32).T).astype(bf)
    bias = np.ascontiguousarray(np.asarray(b_out, dtype=np.float32).reshape(1, DIM))
    x = np.asarray(x, dtype=np.float32)
    return [
        {
            "xt": np.ascontiguousarray(x[b].T).astype(bf),
            "wqkvt": wqkvt,
            "woutt": woutt,
            "bias": bias,
        }
        for b in range(N_CORES)
    ]


def kernel(x, w_qkv, w_out, b_out):
    nc = _get_nc()
    in_maps = make_in_maps(x, w_qkv, w_out, b_out)
    res = run_bass_kernel_spmd(nc, in_maps, core_ids=list(range(N_CORES)))
    return np.stack(
        [res.results[b]["out"] for b in range(N_CORES)], axis=0
    ).astype(np.float32)


# revision 21
# speedup vs baseline: 1.0312x; 1.0312x over previous
"""Trainium2 Bass kernel for nn_Attention_2851858284976.

Dense transformer attention block, b=8 n=1024 dim=1024 heads=16.
Sharding: pure data parallel — one batch element per NeuronCore (8 cores).

Per-core math (batch element x of shape (n, dim)):
  Y = x @ w_qkv^T                              (n, 3*dim)
  Z = Y.reshape(49152, 64)   # raw reshape: rows are (token, col-block) pairs
  Q = Z[0:16384], K = Z[16384:32768], V = Z[32768:49152], each (16, 1024, 64)
  per head: P^T = exp(scale * K_h @ Q_h^T)     (softmax along the partition axis)
            [O^T; Zs*64] = [V_h | 1*64]^T @ P^T  (ones cols replicate the denom)
            oT_h = O^T * (1/Zs)
  out = (oT stacked).T @ w_out^T + b_out

Q/K SBUF layout: Z row r = 48*i + c0 decomposes as c0 = 16*a + b (a<3,
b<16), r = 16*T + b with T = 3*i + a.  Heads are 1024 = 64*16 tokens, so
head h's tokens t = 16*u + b map to a contiguous T-run [64h, 64h+64) x all
b.  QKb[d, T, b] (b innermost) therefore gives:
  - contiguous head windows with IDENTITY token order (col = 16*dT + b);
  - phase-1A scatters that write [i, b-run] blocks (4 bf16 at 4B step)
    instead of 96B-strided single elements.

Schedule: the exp stream on ACT (128 x ~1.15us) is the second-largest
engine load after the PE, so phase 1 is split to start it early:
  A-sweep tb0 (i<384; all Q + K of heads 0-1) -> B-it5 (V heads 0-3) ->
  consume loop from head 0 with the REST of phase 1 (A tb1/tb2 2-block
  groups, B it6/it7) spliced between steps.  PSUM: sps 2x2 banks, ops 2
  banks (tags reused each head; [O;Z] spilled to SBUF f32 in one copy to
  free them), leaving 2 banks for the spliced phase-1 psums.
"""
import numpy as np
import ml_dtypes

import concourse.bass as bass
import concourse.mybir as mybir
from concourse import bacc
from concourse.tile import TileContext
from concourse.bass_utils import run_bass_kernel_spmd

N_CORES = 8
N = 1024          # tokens
DIM = 1024
E3 = 3 * DIM      # qkv projection width
H = 16            # heads
HD = 64           # head dim
SCALE = HD ** -0.5
TSPAN = 2049      # T = 3*i + a, i < 683 -> T in [0, 2049)
VROWS = 48 * 342  # 16416: V Z-rows (16384) plus 32 slack
AHEAD = 6         # produce-ahead depth (pt queue)

F32 = mybir.dt.float32
BF = mybir.dt.bfloat16
FT = mybir.ActivationFunctionType


def build():
    nc = bacc.Bacc("TRN2", target_bir_lowering=False, num_devices=N_CORES)
    xt = nc.declare_dram_parameter("xt", [DIM, N], BF, isOutput=False)
    wqkvt = nc.declare_dram_parameter("wqkvt", [DIM, E3], BF, isOutput=False)
    woutt = nc.declare_dram_parameter("woutt", [DIM, DIM], BF, isOutput=False)
    bias = nc.declare_dram_parameter("bias", [1, DIM], F32, isOutput=False)
    outp = nc.declare_dram_parameter("out", [N, DIM], F32, isOutput=True)

    with TileContext(nc) as tc:
        with tc.tile_pool(name="dram", bufs=1, space="DRAM") as dpool, \
             tc.tile_pool(name="singles", bufs=1) as singles:
            # packed V buffer: flat row q = Z row 32768+q = v of head q//1024
            vbuf = dpool.tile([VROWS, HD], BF)
            vb3 = vbuf.rearrange("(a c) d -> a c d", c=48)   # (342, 48, 64)

            oT = singles.tile([128, 8, N], BF)    # [64*(h%2)+dd, h//2, i]
            biasrep = singles.tile([128, DIM], F32)
            vh0 = singles.tile([128, 8, 2 * HD], BF)
            vh1 = singles.tile([128, 8, 2 * HD], BF)
            vh2 = singles.tile([128, 8, 2 * HD], BF)
            vh3 = singles.tile([128, 8, 2 * HD], BF)
            vhs = [vh0, vh1, vh2, vh3]
            WOT = singles.tile([128, 8, DIM], BF)

            with tc.tile_pool(name="qk", bufs=1) as qkpool:
                # QKb[d, T, b] = Z[16*T + b, d]  (Q: T<1024, K: 1024<=T<2048)
                QKb = qkpool.tile([64, TSPAN, 16], BF)
                # scatter view: T = 3*i + a, b = 2*b2 + dl
                QKs = QKb.rearrange("p (i a) (b2 dl) -> p i a b2 dl",
                                    a=3, dl=2)

                def qt_sl(h, ic):
                    w = QKb[0:64, 64 * h + 32 * ic:64 * h + 32 * ic + 32, :]
                    return w.rearrange("p T b -> p (T b)")

                def kt_sl(h, jt):
                    w = QKb[0:64, 1024 + 64 * h + 8 * jt:
                            1024 + 64 * h + 8 * jt + 8, :]
                    return w.rearrange("p T b -> p (T b)")

                with tc.tile_pool(name="p1", bufs=1) as p1:
                    XT = p1.tile([128, 8, N], BF)
                    # WT[:, kt, a, e'] = wqkvt rows kt*128.., col a*1024+e'
                    WT = p1.tile([128, 8, 3, N], BF)
                    for kt in range(8):
                        nc.sync.dma_start(
                            out=XT[:, kt, :], in_=xt[kt * 128:(kt + 1) * 128, :])
                    for a in range(3):
                        for kh in range(2):
                            nc.sync.dma_start(
                                out=WT[:, 4 * kh:4 * kh + 4, a, :],
                                in_=wqkvt[kh * 512:(kh + 1) * 512,
                                          a * N:(a + 1) * N].rearrange(
                                              "(kt p) e -> p kt e", p=128))
                    nc.sync.dma_start(out=biasrep,
                                      in_=bias[:].to_broadcast((128, DIM)))
                    nc.sync.dma_start(
                        out=WOT, in_=woutt[:].rearrange("(a p) e -> p a e", p=128))
                    # [V | ones*64] stationaries for the PV matmul; ones half
                    # replicates the softmax denominator on out rows 64-127.
                    for v in vhs:
                        nc.vector.memset(v, 1.0)

                    # sweep boundaries: tb0 covers all Q (i<=341) + K heads
                    # 0-1 (i<=383); tb1 unlocks K heads 2-7 (i<=511); tb2
                    # the rest.
                    SWEEPS = [(0, 384), (384, 534), (534, 683)]

                    def a_group_mms(ps, nblk, a, m0, i_lo, i_hi):
                        cnt = i_hi - i_lo
                        for mm in range(nblk):
                            m = m0 + mm
                            for kt in range(8):
                                nc.tensor.matmul(
                                    ps[:, mm, 0:cnt],
                                    lhsT=WT[:, kt, a, m * 128:(m + 1) * 128],
                                    rhs=XT[:, kt, i_lo:i_hi],
                                    start=(kt == 0), stop=(kt == 7))

                    def a_group_scatter(ps, nblk, a, m0, i_lo, i_hi):
                        cnt = i_hi - i_lo
                        for dl in range(2):  # c0 parity
                            src = ps[64 * dl:64 * dl + 64, :, 0:cnt]
                            src = src.rearrange("p m i -> p i m")
                            dst = QKs[0:64, i_lo:i_hi, a, m0:m0 + nblk, dl]
                            # DVE only: keep the ACT queue free for exps
                            nc.vector.tensor_copy(dst, src)

                    # ---------- phase 1A sweep tb0 (i < 384) --------------
                    with tc.tile_pool(name="psA0", bufs=2, space="PSUM") as psA0:
                        i_lo, i_hi = SWEEPS[0]
                        for a in range(3):
                            for mh in range(2):
                                ps4 = psA0.tile([128, 4, 512], F32)
                                a_group_mms(ps4, 4, a, 4 * mh, i_lo, i_hi)
                                a_group_scatter(ps4, 4, a, 4 * mh, i_lo, i_hi)

                    # ---------- phase 1B tiles ----------------------------
                    def emit_b_tile(bit, ec, pBpool, psBpool):
                        ps = psBpool.tile([128, 512], F32)
                        for kt in range(8):
                            nc.tensor.matmul(
                                ps,
                                lhsT=XT[:, kt, bit * 128:(bit + 1) * 128],
                                rhs=WT[:, kt, (ec * 512) // N,
                                       (ec * 512) % N:(ec * 512) % N + 512],
                                start=(kt == 0), stop=(kt == 7))
                        st = pBpool.tile([128, 8, HD], BF)
                        nc.vector.tensor_copy(
                            st, ps.rearrange("p (b d) -> p b d", d=HD))
                        # V rows: q = 48*T + c0 - 32768
                        if ec <= 3:
                            plo = 43 if bit == 5 else 0
                            nc.sync.dma_start(
                                out=vb3[bit * 128 + plo - 683:
                                        (bit + 1) * 128 - 683,
                                        ec * 8 + 16: ec * 8 + 24, :],
                                in_=st[plo:128, :, :])
                        else:
                            plo = 42 if bit == 5 else 0
                            nc.sync.dma_start(
                                out=vb3[bit * 128 + plo - 682:
                                        (bit + 1) * 128 - 682,
                                        (ec - 4) * 8: (ec - 4) * 8 + 8, :],
                                in_=st[plo:128, :, :])

                    def load_v(h):
                        nc.sync.dma_start(
                            out=vhs[h % 4][:, :, 0:HD],
                            in_=vbuf[h * N:(h + 1) * N, :].rearrange(
                                "(t p) d -> p t d", p=128))

                    with tc.tile_pool(name="pB", bufs=2) as pB:
                        # ------ phase 2 + spliced phase-1 leftovers -------
                        with tc.tile_pool(name="pt", bufs=AHEAD) as ptpool, \
                             tc.tile_pool(name="sp", bufs=1) as sppool, \
                             tc.tile_pool(name="rz", bufs=1) as rzpool, \
                             tc.tile_pool(name="sps", bufs=2,
                                          space="PSUM") as spsum:
                            steps = [(h, jt) for h in range(H) for jt in range(8)]

                            def produce(h, jt):
                                sps = spsum.tile([128, 2, 512], F32, tag="sps")
                                warm = vhs[(h + 2) % 4]
                                # HAM warm-keeper; overwritten by the real
                                # scores matmul (start=True).
                                nc.tensor.matmul(
                                    sps[0:128, 0, 0:128],
                                    lhsT=warm[:, 0, :], rhs=warm[:, 0, :],
                                    start=True, stop=True)
                                for ic in range(2):
                                    nc.tensor.matmul(
                                        sps[:, ic, :],
                                        lhsT=kt_sl(h, jt),
                                        rhs=qt_sl(h, ic),
                                        start=True, stop=True)
                                pt = ptpool.tile([128, 2, 512], BF, tag="pt")
                                nc.scalar.activation(pt, sps, FT.Exp, scale=SCALE)
                                return pt

                            # emission-order gates (the tile framework can
                            # only order reads after ALREADY-EMITTED writes):
                            # produce(h) needs its K sweep scattered; V loads
                            # for h>=4 need the B it6/it7 stores emitted.
                            state = {"next_p": 0, "limit": 16, "ops": None,
                                     "pend_v": []}
                            pts = []

                            def fill_pts():
                                while (state["next_p"] < min(state["limit"],
                                                             len(steps))
                                       and len(pts) < AHEAD):
                                    pts.append(produce(*steps[state["next_p"]]))
                                    state["next_p"] += 1

                            def flush_v(s):
                                keep = []
                                for thr, hv in state["pend_v"]:
                                    if s >= thr:
                                        load_v(hv)
                                    else:
                                        keep.append((thr, hv))
                                state["pend_v"] = keep

                            def consume_step(s):
                                h, jt = steps[s]
                                po, hf = 64 * (h % 2), h // 2
                                flush_v(s)
                                if jt == 0:
                                    # 2 tags reused by every head: frees 2
                                    # PSUM banks for the spliced phase-1 work
                                    ops0 = state["opsum"].tile([128, 512], F32,
                                                               tag="ops0")
                                    ops1 = state["opsum"].tile([128, 512], F32,
                                                               tag="ops1")
                                    state["ops"] = (ops0, ops1)
                                ops = state["ops"]
                                assert pts, f"pt queue empty at step {s}"
                                pt_cur = pts.pop(0)
                                fill_pts()
                                for ic in range(2):
                                    nc.tensor.matmul(
                                        ops[ic],
                                        lhsT=vhs[h % 4][:, jt, :],
                                        rhs=pt_cur[:, ic, :],
                                        start=(jt == 0), stop=(jt == 7),
                                        skip_group_check=True)
                                if jt == 7 and h + 4 < H:
                                    hv = h + 4
                                    thr = 0 if hv < 4 else (30 if hv < 10 else 36)
                                    if s >= thr:
                                        load_v(hv)
                                    else:
                                        state["pend_v"].append((thr, hv))
                                if jt == 7:
                                    # spill [O; Z] to SBUF to free the ops
                                    # tags fast; recip+mul run off the
                                    # critical path.  The custom-DVE recip
                                    # needs UNSHIFTED partitions (regular
                                    # copies handle the 64->0 shift).
                                    spz = sppool.tile([64, 2, 512], F32,
                                                      tag=f"spz{h % 2}")
                                    spo = sppool.tile([64, 2, 512], F32,
                                                      tag=f"spo{h % 2}")
                                    for ic in range(2):
                                        nc.vector.tensor_copy(
                                            spz[:, ic, :], ops[ic][64:128, :])
                                        nc.vector.tensor_copy(
                                            spo[:, ic, :], ops[ic][0:64, :])
                                    rzs = rzpool.tile([64, 2, 512], F32,
                                                      tag="rzs")
                                    nc.vector.reciprocal_approx_fast(rzs, spz)
                                    nc.vector.tensor_mul(
                                        oT[po:po + 64, hf, :].rearrange(
                                            "p (i c) -> p i c", c=512),
                                        spo, rzs)

                            # pre-produce h0 j0-5 right after A-tb0 so the
                            # ACT exp stream starts before B5's PE work.
                            fill_pts()

                            # B-it5: V heads 0-3 live in its rows.
                            with tc.tile_pool(name="psB5", bufs=4,
                                              space="PSUM") as psB5:
                                for ec in range(6):
                                    emit_b_tile(5, ec, pB, psB5)
                            for hv in range(4):
                                load_v(hv)

                            s = 0
                            with tc.tile_pool(name="ops", bufs=1,
                                              space="PSUM") as opsum2:
                                state["opsum"] = opsum2
                                # A sweeps tb1/tb2 as 2-block groups (2 psum
                                # banks, bufs=1) spliced one per consume step.
                                # tb1 done -> K heads <=8 valid; tb2 -> all.
                                with tc.tile_pool(name="psA1", bufs=1,
                                                  space="PSUM") as psA1:
                                    for tb in (1, 2):
                                        i_lo, i_hi = SWEEPS[tb]
                                        for a in range(3):
                                            for mp in range(4):
                                                consume_step(s); s += 1
                                                ps2 = psA1.tile(
                                                    [128, 2, 512], F32)
                                                a_group_mms(ps2, 2, a, 2 * mp,
                                                            i_lo, i_hi)
                                                a_group_scatter(
                                                    ps2, 2, a, 2 * mp,
                                                    i_lo, i_hi)
                                        state["limit"] = 72 if tb == 1 else 128
                                        fill_pts()
                                with tc.tile_pool(name="psB2", bufs=2,
                                                  space="PSUM") as psB2:
                                    for bit in (6, 7):
                                        for ec in range(6):
                                            consume_step(s); s += 1
                                            emit_b_tile(bit, ec, pB, psB2)
                                while s < len(steps):
                                    consume_step(s); s += 1

                # ---------- phase 3: out = oT.T @ w_out^T + b ----------
                with tc.tile_pool(name="p3st", bufs=4) as p3st, \
                     tc.tile_pool(name="ps3", bufs=4, space="PSUM") as ps3p:
                    for it in range(8):
                        for ec in range(2):
                            rps = ps3p.tile([128, 512], F32)
                            for ct in range(8):
                                nc.tensor.matmul(
                                    rps,
                                    lhsT=oT[:, ct, it * 128:(it + 1) * 128],
                                    rhs=WOT[:, ct, ec * 512:(ec + 1) * 512],
                                    start=(ct == 0), stop=(ct == 7))
                            ost = p3st.tile([128, 512], F32)
                            nc.vector.tensor_add(
                                ost, rps, biasrep[:, ec * 512:(ec + 1) * 512])
                            nc.sync.dma_start(
                                out=outp[it * 128:(it + 1) * 128,
                                         ec * 512:(ec + 1) * 512],
                                in_=ost)

    nc.finalize()
    return nc


_CACHE = {}


def _get_nc():
    if "nc" not in _CACHE:
        _CACHE["nc"] = build()
    return _CACHE["nc"]


def make_in_maps(x, w_qkv, w_out, b_out):
    bf = ml_dtypes.bfloat16
    wqkvt = np.ascontiguousarray(np.asarray(w_qkv, dtype=np.float32).T).astype(bf)
    woutt = np.ascontiguousarray(np.asarray(w_out, dtype=np.float32).T).astype(bf)
    bias = np.ascontiguousarray(np.asarray(b_out, dtype=np.float32).reshape(1, DIM))
    x = np.asarray(x, dtype=np.float32)
    return [
        {
            "xt": np.ascontiguousarray(x[b].T).astype(bf),
            "wqkvt": wqkvt,
            "woutt": woutt,
            "bias": bias,
        }
        for b in range(N_CORES)
    ]


def kernel(x, w_qkv, w_out, b_out):
    nc = _get_nc()
    in_maps = make_in_maps(x, w_qkv, w_out, b_out)
    res = run_bass_kernel_spmd(nc, in_maps, core_ids=list(range(N_CORES)))
    return np.stack(
        [res.results[b]["out"] for b in range(N_CORES)], axis=0
    ).astype(np.float32)
